# revision 1
# baseline (speedup 1.0000x reference)
"""Trainium2 Bass kernel: causal multi-head attention with LoRA (B=2, T=2048,
C=1024, 16 heads, r=16), SPMD across 8 NeuronCores.

Sharding: core = (batch, head-group-of-4). QKV + attention are fully local per
core (weights pre-sliced per head group on host); the output projection is
computed as a partial sum over each core's 256 y-features and reduced on host.

Matmuls run in float32r (fp32 storage, single-pass PE mode, 4x faster than
plain fp32). Scores are computed transposed (S^T: k on partitions, q on free)
so softmax needs no on-chip transposes: exp via ScalarE (no max subtraction --
scores are O(1) for this problem's 0.02-scaled weights), denominators from an
appended ones-column in V (row 64 of the AV accumulation), normalization via a
PE-broadcast of the denominator row and a vector reciprocal.
"""
import os
import sys

sys.path.insert(0, "/opt/trn_rl_repo")

import numpy as np

import concourse.bass as bass  # noqa: F401
import concourse.bacc as bacc
import concourse.tile as tile
import concourse.mybir as mybir
from concourse.bass_utils import run_bass_kernel_spmd

B, T, C = 2, 2048, 1024
H, HD = 16, 64
R = 16
LORA_SCALE = 1.0 / R
N_CORES = 8
GPB = N_CORES // B          # core groups per batch = 4
HPC = H // GPB              # heads per core = 4
CI = HPC * HD               # per-core y features = 256
P = 128
T5 = T // 512               # 4  (512-wide t tiles)
T1 = T // P                 # 16 (128-wide t tiles)
CT = C // P                 # 8  (128-wide c tiles)
FQK = 2 * HPC * HD // P     # 4  (128-wide qk feature tiles: f0,f1=q f2,f3=k)
F32 = mybir.dt.float32
MM = mybir.dt.float32r
BF16 = mybir.dt.bfloat16

LAST_RESULTS = None
_CACHE = {}


def build():
    nc = bacc.Bacc("TRN2", target_bir_lowering=False, debug=False,
                   num_devices=N_CORES)

    xt_d = nc.dram_tensor("xt", [C, T], MM, kind="ExternalInput").ap()
    wqk_d = nc.dram_tensor("wqk", [C, 2 * CI], MM, kind="ExternalInput").ap()
    wv_d = nc.dram_tensor("wv", [C, CI], MM, kind="ExternalInput").ap()
    bqk_d = nc.dram_tensor("bqk", [2 * CI, 1], F32, kind="ExternalInput").ap()
    laa_d = nc.dram_tensor("laa", [C, R], MM, kind="ExternalInput").ap()
    lbaqk_d = nc.dram_tensor("lbaqk", [R, 2 * CI], MM, kind="ExternalInput").ap()
    lbav_d = nc.dram_tensor("lbav", [R + 1, CI], MM, kind="ExternalInput").ap()
    wp_d = nc.dram_tensor("wp", [CI, C], MM, kind="ExternalInput").ap()
    lap_d = nc.dram_tensor("lap", [CI, R], MM, kind="ExternalInput").ap()
    lbp_d = nc.dram_tensor("lbp", [R, C], MM, kind="ExternalInput").ap()
    bp_d = nc.dram_tensor("bp", [C, 1], F32, kind="ExternalInput").ap()
    masks_d = nc.dram_tensor("masks", [P, 896], BF16, kind="ExternalInput").ap()
    onesr_d = nc.dram_tensor("onesr", [1, T], MM, kind="ExternalInput").ap()
    onesc_d = nc.dram_tensor("onesc", [1, HD], MM, kind="ExternalInput").ap()
    vones_d = nc.dram_tensor("vones", [P, T1 * HPC], BF16, kind="ExternalInput").ap()
    out_d = nc.dram_tensor("out", [C, T], F32, kind="ExternalOutput").ap()

    with tile.TileContext(nc) as tc:
        with (
            tc.tile_pool(name="const", bufs=1) as cp,
            tc.tile_pool(name="work", bufs=2) as wk,
            tc.tile_pool(name="att", bufs=4) as ap_,
            tc.tile_pool(name="ps", bufs=3, space="PSUM") as ps,
            tc.tile_pool(name="pss", bufs=2, space="PSUM") as pss,
            tc.tile_pool(name="psav", bufs=3, space="PSUM") as psav,
        ):
            # ---- resident SBUF tensors -------------------------------------
            xt_sb = cp.tile([P, CT, T], MM)             # x^T            64 KB
            wqk_sb = cp.tile([P, CT, FQK, P], MM)       # W_qk^T         16 KB
            wv_sb = cp.tile([P, CT, CI], MM)            # W_v^T           8 KB
            laa_sb = cp.tile([P, CT, R], MM)            # A_attn^T       .5 KB
            lbaqk_sb = cp.tile([R, FQK, P], MM)         # B_attn qk^T/16  2 KB
            lbav_sb = cp.tile([R + 1, CI], MM)          # [B_attn v/16;bv] 1KB
            wp_sb = cp.tile([P, 2, CT, P], MM)          # W_proj^T slice  8 KB
            lap_sb = cp.tile([P, 2, R], MM)             # A_proj^T slice  tiny
            lbp_sb = cp.tile([R, CT, P], MM)            # B_proj^T/16     4 KB
            bqk_sb = cp.tile([P, FQK], F32)
            bp_sb = cp.tile([P, CT], F32)
            qk_sb = cp.tile([P, FQK, T], MM)            # q,k feature-major 32 KB
            v_sb = cp.tile([P, T1, HPC, HD + 1], BF16)  # v natural + ones 8.1 KB
            u_sb = cp.tile([R + 1, T], MM)              # lora-u + ones row 8 KB
            yt_sb = cp.tile([P, 2, T], MM)              # y^T (ci-major)  16 KB
            up_sb = cp.tile([R, T], MM)                 # proj-lora u      8 KB
            masks = cp.tile([P, 896], BF16)             # causal masks   1.75 KB
            ones_sb = cp.tile([HD + 1, HD], MM)         # PE-bcast stationary

            # ---- input DMAs ------------------------------------------------
            for c in range(CT):
                nc.sync.dma_start(out=xt_sb[:, c, :], in_=xt_d[c * P:(c + 1) * P, :])
                for f in range(FQK):
                    nc.sync.dma_start(out=wqk_sb[:, c, f, :],
                                      in_=wqk_d[c * P:(c + 1) * P, f * P:(f + 1) * P])
                nc.sync.dma_start(out=wv_sb[:, c, :], in_=wv_d[c * P:(c + 1) * P, :])
                nc.sync.dma_start(out=laa_sb[:, c, :], in_=laa_d[c * P:(c + 1) * P, :])
            for f in range(FQK):
                nc.sync.dma_start(out=lbaqk_sb[:, f, :],
                                  in_=lbaqk_d[:, f * P:(f + 1) * P])
                nc.sync.dma_start(out=bqk_sb[:, f:f + 1],
                                  in_=bqk_d[f * P:(f + 1) * P, :])
            nc.sync.dma_start(out=lbav_sb[:], in_=lbav_d[:])
            for ci in range(2):
                for co in range(CT):
                    nc.sync.dma_start(out=wp_sb[:, ci, co, :],
                                      in_=wp_d[ci * P:(ci + 1) * P, co * P:(co + 1) * P])
                nc.sync.dma_start(out=lap_sb[:, ci, :], in_=lap_d[ci * P:(ci + 1) * P, :])
            for co in range(CT):
                nc.sync.dma_start(out=lbp_sb[:, co, :], in_=lbp_d[:, co * P:(co + 1) * P])
                nc.sync.dma_start(out=bp_sb[:, co:co + 1], in_=bp_d[co * P:(co + 1) * P, :])
            nc.sync.dma_start(out=masks[:], in_=masks_d[:])
            nc.sync.dma_start(out=u_sb[R:R + 1, :], in_=onesr_d[:])
            nc.sync.dma_start(out=ones_sb[HD:HD + 1, :], in_=onesc_d[:])
            nc.sync.dma_start(out=v_sb[:, :, :, HD:HD + 1], in_=vones_d[:])

            # ---- phase 1: u = A_attn @ x^T  (R x T) ------------------------
            for j in range(T5):
                pu = ps.tile([R, 512], F32, tag="ps")
                for c in range(CT):
                    nc.tensor.matmul(pu[:], laa_sb[:, c, :],
                                     xt_sb[:, c, j * 512:(j + 1) * 512],
                                     start=(c == 0), stop=(c == CT - 1))
                nc.scalar.copy(u_sb[0:R, j * 512:(j + 1) * 512], pu[:])

            # ---- phase 2: qk^T = W_qk @ x^T + B_qk @ u + bias --------------
            # f order 0,2,1,3 so heads 0/1 (need f0+f2) unblock attention
            # early; j-pairs share one weight load (stationary reuse).
            for f in (0, 2, 1, 3):
                for j0 in range(0, T5, 2):
                    pqs = [ps.tile([P, 512], F32, tag="ps", name=f"pq{f}_{j0}_{dj}")
                           for dj in range(2)]
                    for c in range(CT):
                        for dj in range(2):
                            j = j0 + dj
                            nc.tensor.matmul(pqs[dj][:], wqk_sb[:, c, f, :],
                                             xt_sb[:, c, j * 512:(j + 1) * 512],
                                             start=(c == 0), stop=False)
                    for dj in range(2):
                        j = j0 + dj
                        nc.tensor.matmul(pqs[dj][:], lbaqk_sb[:, f, :],
                                         u_sb[0:R, j * 512:(j + 1) * 512],
                                         start=False, stop=True)
                        nc.scalar.activation(qk_sb[:, f, j * 512:(j + 1) * 512],
                                             pqs[dj][:],
                                             mybir.ActivationFunctionType.Identity,
                                             bias=bqk_sb[:, f:f + 1])

            # ---- phase 3: V natural = x @ W_v^T + u^T @ B_v^T (+bias) ------
            for i in range(T1):
                pv = ps.tile([P, HPC, HD], F32, tag="ps")
                for c in range(CT):
                    nc.tensor.matmul(pv[:], xt_sb[:, c, i * P:(i + 1) * P],
                                     wv_sb[:, c, :],
                                     start=(c == 0), stop=False)
                nc.tensor.matmul(pv[:], u_sb[:, i * P:(i + 1) * P],
                                 lbav_sb[:], start=False, stop=True)
                nc.scalar.copy(v_sb[:, i, :, 0:HD], pv[:])

            # ---- phase 4: attention per head -------------------------------
            # The normalize chain for unit (h, j) is emitted two units later
            # so its cross-engine deps are long satisfied when the static PE
            # program reaches the broadcast matmul (no PE stall -> HAM warm).
            pending = []

            def flush_one():
                pav, h, j = pending.pop(0)
                # normalize: y^T = yu^T * (1/D), D broadcast via PE
                bsb = wk.tile([HD + 1, 512], MM, tag="bsb")
                nc.scalar.copy(bsb[HD:HD + 1, :], pav[HD:HD + 1, :])
                pb = ps.tile([HD, 512], F32, tag="ps")
                nc.tensor.matmul(pb[:], ones_sb[HD:HD + 1, :],
                                 bsb[HD:HD + 1, :], start=True, stop=True)
                rsb = wk.tile([HD, 512], F32, tag="rsb")
                nc.vector.reciprocal(rsb[:], pb[:])
                if h % 2 == 0:
                    nc.vector.tensor_tensor(
                        yt_sb[0:HD, h // 2, j * 512:(j + 1) * 512],
                        pav[0:HD, :], rsb[:], mybir.AluOpType.mult)
                else:
                    tsb = wk.tile([HD, 512], MM, tag="tsb")
                    nc.vector.tensor_tensor(tsb[:], pav[0:HD, :], rsb[:],
                                            mybir.AluOpType.mult)
                    nc.sync.dma_start(
                        out=yt_sb[HD:P, h // 2, j * 512:(j + 1) * 512],
                        in_=tsb[:])

            for h in range(HPC):
                pq_base = (h % 2) * HD
                fq = h // 2
                fk = 2 + h // 2
                for j in range(T5):
                    ni = 4 * j + 4  # causal k-tile count for this q block
                    pav = psav.tile([HD + 1, 512], F32, tag="psav")
                    for i in range(ni):
                        pst = pss.tile([P, 512], F32, tag="pss")
                        kt = qk_sb[pq_base:pq_base + HD, fk, i * P:(i + 1) * P]
                        qt = qk_sb[pq_base:pq_base + HD, fq, j * 512:(j + 1) * 512]
                        nc.tensor.matmul(pst[:], kt, qt, start=True, stop=True)
                        at = ap_.tile([P, 512], BF16, tag="att")
                        nc.scalar.activation(at[:], pst[:],
                                             mybir.ActivationFunctionType.Exp,
                                             scale=0.125)
                        a = i - 4 * j
                        if a >= 0:
                            nc.vector.tensor_tensor(
                                at[:], at[:],
                                masks[:, 384 - 128 * a:896 - 128 * a],
                                mybir.AluOpType.mult)
                        nc.tensor.matmul(pav[:], v_sb[:, i, h, :], at[:],
                                         start=(i == 0), stop=(i == ni - 1))
                        if i == 1 and len(pending) >= 2:
                            flush_one()
                    pending.append((pav, h, j))
            while pending:
                flush_one()

            # ---- phase 5: up = A_proj_slice @ y^T --------------------------
            for j in range(T5):
                pu = ps.tile([R, 512], F32, tag="ps")
                for ci in range(2):
                    nc.tensor.matmul(pu[:], lap_sb[:, ci, :],
                                     yt_sb[:, ci, j * 512:(j + 1) * 512],
                                     start=(ci == 0), stop=(ci == 1))
                nc.scalar.copy(up_sb[:, j * 512:(j + 1) * 512], pu[:])

            # ---- phase 6: out^T partial = W_p^T-slice @ y^T + B_p @ up -----
            for co in range(CT):
                for j in range(T5):
                    po = ps.tile([P, 512], F32, tag="ps")
                    for ci in range(2):
                        nc.tensor.matmul(po[:], wp_sb[:, ci, co, :],
                                         yt_sb[:, ci, j * 512:(j + 1) * 512],
                                         start=(ci == 0), stop=False)
                    nc.tensor.matmul(po[:], lbp_sb[:, co, :],
                                     up_sb[:, j * 512:(j + 1) * 512],
                                     start=False, stop=True)
                    oq = wk.tile([P, 512], F32, tag="oq")
                    nc.vector.tensor_scalar_add(oq[:], po[:],
                                                bp_sb[:, co:co + 1])
                    nc.sync.dma_start(
                        out=out_d[co * P:(co + 1) * P, j * 512:(j + 1) * 512],
                        in_=oq[:])

    nc.compile()
    return nc


def _shard_inputs(x, w_attn, b_attn, lora_a_attn, lora_b_attn, w_proj, b_proj,
                  lora_a_proj, lora_b_proj):
    f32 = np.float32
    x = np.asarray(x, f32)
    w_attn = np.asarray(w_attn, f32)
    b_attn = np.asarray(b_attn, f32)
    lora_a_attn = np.asarray(lora_a_attn, f32)
    lora_b_attn = np.asarray(lora_b_attn, f32)
    w_proj = np.asarray(w_proj, f32)
    b_proj = np.asarray(b_proj, f32)
    lora_a_proj = np.asarray(lora_a_proj, f32)
    lora_b_proj = np.asarray(lora_b_proj, f32)

    laa_t = np.ascontiguousarray(lora_a_attn.T)               # (C, R)
    lbp = np.ascontiguousarray((lora_b_proj * LORA_SCALE).T)  # (R, C)
    import ml_dtypes
    bf16 = ml_dtypes.bfloat16
    # masks[p, z] = 1.0 if z >= p + 384 else 0.0
    pp, zz = np.meshgrid(np.arange(P), np.arange(896), indexing="ij")
    masks = (zz >= pp + 384).astype(bf16)
    onesr = np.ones((1, T), f32)
    onesc = np.ones((1, HD), f32)
    vones = np.ones((P, T1 * HPC), bf16)
    in_maps = []
    for core in range(N_CORES):
        b = core // GPB
        heads = [(core % GPB) * HPC + k for k in range(HPC)]
        q_idx = np.concatenate([np.arange(h * HD, (h + 1) * HD) for h in heads])
        k_idx = q_idx + C
        v_idx = q_idx + 2 * C
        qk_idx = np.concatenate([q_idx, k_idx])
        wqk_t = np.ascontiguousarray(w_attn[qk_idx].T)        # (C, 512)
        wv_t = np.ascontiguousarray(w_attn[v_idx].T)          # (C, 256)
        bqk = np.ascontiguousarray(b_attn[qk_idx][:, None])   # (512, 1)
        bv = b_attn[v_idx]
        lbaqk = np.ascontiguousarray((lora_b_attn[qk_idx] * LORA_SCALE).T)
        lbav = np.concatenate(
            [(lora_b_attn[v_idx] * LORA_SCALE).T, bv[None, :]], 0)  # (R+1, 256)
        wp_t = np.ascontiguousarray(w_proj[:, q_idx].T)       # (256, C)
        lap_t = np.ascontiguousarray(lora_a_proj[:, q_idx].T)  # (256, R)
        bp = b_proj[:, None] if core % GPB == 0 else np.zeros((C, 1), f32)
        in_maps.append({
            "xt": np.ascontiguousarray(x[b].T),
            "wqk": wqk_t, "wv": wv_t, "bqk": bqk,
            "laa": laa_t, "lbaqk": lbaqk,
            "lbav": np.ascontiguousarray(lbav),
            "wp": wp_t, "lap": lap_t, "lbp": lbp,
            "bp": np.ascontiguousarray(bp),
            "masks": masks, "onesr": onesr, "onesc": onesc, "vones": vones,
        })
    return in_maps


def kernel(x, w_attn, b_attn, lora_a_attn, lora_b_attn, w_proj, b_proj,
           lora_a_proj, lora_b_proj, n_head):
    global LAST_RESULTS
    assert int(n_head) == H
    if "nc" not in _CACHE:
        _CACHE["nc"] = build()
    nc = _CACHE["nc"]
    in_maps = _shard_inputs(x, w_attn, b_attn, lora_a_attn, lora_b_attn,
                            w_proj, b_proj, lora_a_proj, lora_b_proj)
    res = run_bass_kernel_spmd(
        nc, in_maps, core_ids=list(range(N_CORES)),
        trace=bool(os.environ.get("BASS_KERNEL_TRACE")))
    LAST_RESULTS = res
    out = np.zeros((B, C, T), np.float32)
    for core in range(N_CORES):
        out[core // GPB] += res.results[core]["out"]
    return np.ascontiguousarray(out.transpose(0, 2, 1))



# revision 5
# speedup vs baseline: 1.7996x; 1.7996x over previous
"""Trainium2 Bass kernel: causal multi-head attention with LoRA (B=2, T=2048,
C=1024, 16 heads, r=16), SPMD across 8 NeuronCores.

Sharding: core = (batch, head-group-of-4). QKV + attention are fully local per
core (weights pre-sliced per head group on host); the output projection is
computed as a partial sum over each core's 256 y-features and reduced on host.

v2 design (vs the fp32r baseline):
- All operands bf16 (PSUM accumulation stays fp32). Halves LDWEIGHTS time,
  enables FWL, halves input DMA, and doubles DVE throughput on masks.
- Phase 4 (attention) is software-pipelined: score matmuls for k-tile pair p
  are emitted one step ahead of the AV matmuls for pair p-1, so the PE never
  waits on ScalarE's exp. This keeps the HAM clock gate at K=8/8 (2.4 GHz);
  the baseline idled the PE between score and AV matmuls, which re-throttled
  the PE to 1.2 GHz for the final 300us of the kernel.
- exp is batched over k-tile pairs ([128,1024] per ACTIVATE) to amortize the
  352-cycle ScalarE instruction overhead.
- All biases are folded into matmuls via ones-rows appended to the LoRA-u
  vectors (rank 16 -> 17 contraction), so no separate bias-add passes.
- Softmax normalization: denominator row is PE-broadcast, reciprocal via
  DVE reciprocal_approx_fast (5x faster than reciprocal), one multiply.
- The output projection for query block j-1 is interleaved into the PE stream
  of attention block j, filling the PE slack left by ScalarE-bound exp.
"""
import os
import sys

sys.path.insert(0, "/opt/trn_rl_repo")

import numpy as np

import concourse.bass as bass  # noqa: F401
import concourse.bacc as bacc
import concourse.tile as tile
import concourse.mybir as mybir
from concourse.bass_utils import run_bass_kernel_spmd

B, T, C = 2, 2048, 1024
H, HD = 16, 64
R = 16
LORA_SCALE = 1.0 / R
N_CORES = 8
GPB = N_CORES // B          # core groups per batch = 4
HPC = H // GPB              # heads per core = 4
CI = HPC * HD               # per-core y features = 256
P = 128
T5 = T // 512               # 4  (512-wide query blocks)
T1 = T // P                 # 16 (128-wide key tiles)
CT = C // P                 # 8  (128-wide c tiles)
FQK = 2 * HPC * HD // P     # 4  (128-wide qk feature tiles: f0,f1=q f2,f3=k)
F32 = mybir.dt.float32
BF16 = mybir.dt.bfloat16

LAST_RESULTS = None
_CACHE = {}


def build():
    nc = bacc.Bacc("TRN2", target_bir_lowering=False, debug=False,
                   num_devices=N_CORES)

    xt_d = nc.dram_tensor("xt", [C, T], BF16, kind="ExternalInput").ap()
    wqk_d = nc.dram_tensor("wqk", [C, 2 * CI], BF16, kind="ExternalInput").ap()
    wv_d = nc.dram_tensor("wv", [C, CI], BF16, kind="ExternalInput").ap()
    laa_d = nc.dram_tensor("laa", [C, R], BF16, kind="ExternalInput").ap()
    lbaqk_d = nc.dram_tensor("lbaqk", [R + 1, 2 * CI], BF16, kind="ExternalInput").ap()
    lbav_d = nc.dram_tensor("lbav", [R + 1, CI], BF16, kind="ExternalInput").ap()
    wp_d = nc.dram_tensor("wp", [CI, C], BF16, kind="ExternalInput").ap()
    lap_d = nc.dram_tensor("lap", [CI, R], BF16, kind="ExternalInput").ap()
    lbp_d = nc.dram_tensor("lbp", [R + 1, C], BF16, kind="ExternalInput").ap()
    masks_d = nc.dram_tensor("masks", [P, 896], BF16, kind="ExternalInput").ap()
    onesr_d = nc.dram_tensor("onesr", [1, T], BF16, kind="ExternalInput").ap()
    vones_d = nc.dram_tensor("vones", [P, T1 * HPC], BF16, kind="ExternalInput").ap()
    out_d = nc.dram_tensor("out", [C, T], F32, kind="ExternalOutput").ap()

    with tile.TileContext(nc) as tc:
        with (
            tc.tile_pool(name="const", bufs=1) as cp,
            tc.tile_pool(name="work", bufs=2) as wk,
            tc.tile_pool(name="att", bufs=3) as ap_,
            tc.tile_pool(name="ps", bufs=2, space="PSUM") as ps,
            tc.tile_pool(name="pss", bufs=2, space="PSUM") as pss,
            tc.tile_pool(name="psav", bufs=2, space="PSUM") as psav,
        ):
            # ---- resident SBUF tensors -------------------------------------
            xt_sb = cp.tile([P, CT, T], BF16)            # x^T            32 KB
            wqk_sb = cp.tile([P, CT, FQK, P], BF16)      # W_qk^T          8 KB
            wv_sb = cp.tile([P, CT, CI], BF16)           # W_v^T           4 KB
            laa_sb = cp.tile([P, CT, R], BF16)           # A_attn^T
            lbaqk_sb = cp.tile([R + 1, FQK, P], BF16)    # [B_qk/16; bqk]
            lbav_sb = cp.tile([R + 1, CI], BF16)         # [B_v/16; bv]
            wp_sb = cp.tile([P, 2, CT, P], BF16)         # W_proj^T slice  4 KB
            lap_sb = cp.tile([P, 2, R], BF16)            # A_proj^T slice
            lbp_sb = cp.tile([R + 1, CT, P], BF16)       # [B_p/16; bp]
            qk_sb = cp.tile([P, FQK, T], BF16)           # q,k feat-major 16 KB
            v_sb = cp.tile([P, T1, HPC, HD + 1], BF16)   # v natural + ones
            u_sb = cp.tile([R + 1, T], BF16)             # lora-u + ones row
            up_sb = cp.tile([R + 1, T], BF16)            # proj-lora u + ones
            yt_sb = cp.tile([P, 2, T], BF16)             # y^T (ci-major)  8 KB
            masks = cp.tile([P, 896], BF16)              # causal masks
            onesb = cp.tile([1, HD], BF16)               # bcast stationary

            # ---- input DMAs (xt/laa first: phase 1 is paced by them) -------
            for c in range(CT):
                nc.sync.dma_start(out=xt_sb[:, c, :], in_=xt_d[c * P:(c + 1) * P, :])
                nc.sync.dma_start(out=laa_sb[:, c, :], in_=laa_d[c * P:(c + 1) * P, :])
            nc.sync.dma_start(out=u_sb[R:R + 1, :], in_=onesr_d[:])
            for c in range(CT):
                nc.sync.dma_start(out=wqk_sb[:, c, :, :],
                                  in_=wqk_d[c * P:(c + 1) * P, :])
                nc.sync.dma_start(out=wv_sb[:, c, :], in_=wv_d[c * P:(c + 1) * P, :])
            nc.sync.dma_start(out=lbaqk_sb[:], in_=lbaqk_d[:])
            nc.sync.dma_start(out=lbav_sb[:], in_=lbav_d[:])
            nc.sync.dma_start(out=masks[:], in_=masks_d[:])
            nc.sync.dma_start(out=v_sb[:, :, :, HD:HD + 1], in_=vones_d[:])
            nc.sync.dma_start(out=up_sb[R:R + 1, :], in_=onesr_d[:])
            nc.sync.dma_start(out=onesb[:], in_=onesr_d[0:1, 0:HD])
            for ci in range(2):
                nc.sync.dma_start(out=wp_sb[:, ci, :, :],
                                  in_=wp_d[ci * P:(ci + 1) * P, :])
                nc.sync.dma_start(out=lap_sb[:, ci, :],
                                  in_=lap_d[ci * P:(ci + 1) * P, :])
            nc.sync.dma_start(out=lbp_sb[:], in_=lbp_d[:])

            # ---- phase 1: u = A_attn @ x^T  (R x T) ------------------------
            # c-outer so the PE starts as soon as the first x tile lands.
            for jp in (0, 2):
                pus = [ps.tile([R, 512], F32, tag="ps", name=f"pu{jp}_{d}")
                       for d in range(2)]
                for c in range(CT):
                    for d in range(2):
                        j = jp + d
                        nc.tensor.matmul(pus[d][:], laa_sb[:, c, :],
                                         xt_sb[:, c, j * 512:(j + 1) * 512],
                                         start=(c == 0), stop=(c == CT - 1))
                for d in range(2):
                    j = jp + d
                    nc.scalar.copy(u_sb[0:R, j * 512:(j + 1) * 512], pus[d][:])

            # ---- phase 2: qk^T = W_qk @ x^T + [B_qk;bqk] @ [u;1] -----------
            for f in (0, 2, 1, 3):
                for j in range(T5):
                    pq = ps.tile([P, 512], F32, tag="ps", name=f"pq{f}_{j}")
                    for c in range(CT):
                        nc.tensor.matmul(pq[:], wqk_sb[:, c, f, :],
                                         xt_sb[:, c, j * 512:(j + 1) * 512],
                                         start=(c == 0), stop=False)
                    nc.tensor.matmul(pq[:], lbaqk_sb[:, f, :],
                                     u_sb[:, j * 512:(j + 1) * 512],
                                     start=False, stop=True)
                    nc.scalar.copy(qk_sb[:, f, j * 512:(j + 1) * 512], pq[:])

            # ---- phase 3: V natural = x @ W_v^T + [u;1]^T @ [B_v;bv] -------
            for i in range(T1):
                pv = ps.tile([P, HPC, HD], F32, tag="ps", name=f"pv{i}")
                for c in range(CT):
                    nc.tensor.matmul(pv[:], xt_sb[:, c, i * P:(i + 1) * P],
                                     wv_sb[:, c, :],
                                     start=(c == 0), stop=False)
                nc.tensor.matmul(pv[:], u_sb[:, i * P:(i + 1) * P],
                                 lbav_sb[:], start=False, stop=True)
                nc.scalar.copy(v_sb[:, i, :, 0:HD], pv[:])

            # ---- phase 4 + interleaved phase 5/6 ---------------------------
            flush_queue = []   # units awaiting the normalize chain
            proj_queue = []    # closures: one PE-group of proj work each

            def emit_flush():
                pav, h, j = flush_queue.pop(0)
                # D row (PSUM) -> SBUF bf16, PE-broadcast to 64 partitions,
                # fast reciprocal, then y^T = yu^T * (1/D).
                bsb = wk.tile([1, 512], BF16, tag="bsb", name=f"bsb{h}_{j}")
                nc.vector.tensor_scalar_add(bsb[:], pav[HD:HD + 1, :], 0.0)
                pb = ps.tile([HD, 512], F32, tag="ps", name=f"pb{h}_{j}")
                nc.tensor.matmul(pb[:], onesb[:], bsb[:],
                                 start=True, stop=True)
                rsb = wk.tile([HD, 512], F32, tag="rsb", name=f"rsb{h}_{j}")
                nc.vector.reciprocal_approx_fast(rsb[:], pb[:])
                if h % 2 == 0:
                    nc.vector.tensor_tensor(
                        yt_sb[0:HD, h // 2, j * 512:(j + 1) * 512],
                        pav[0:HD, :], rsb[:], mybir.AluOpType.mult)
                else:
                    tsb = wk.tile([HD, 512], BF16, tag="tsb", name=f"tsb{h}_{j}")
                    nc.vector.tensor_tensor(tsb[:], pav[0:HD, :], rsb[:],
                                            mybir.AluOpType.mult)
                    nc.sync.dma_start(
                        out=yt_sb[HD:P, h // 2, j * 512:(j + 1) * 512],
                        in_=tsb[:])

            def make_proj(j):
                def p5():
                    pu = ps.tile([R, 512], F32, tag="ps", name=f"pu5_{j}")
                    for ci in range(2):
                        nc.tensor.matmul(pu[:], lap_sb[:, ci, :],
                                         yt_sb[:, ci, j * 512:(j + 1) * 512],
                                         start=(ci == 0), stop=(ci == 1))
                    nc.vector.tensor_scalar_add(
                        up_sb[0:R, j * 512:(j + 1) * 512], pu[:], 0.0)
                proj_queue.append(p5)
                for co in range(CT):
                    def p6(co=co):
                        po = ps.tile([P, 512], F32, tag="ps",
                                     name=f"po{j}_{co}")
                        for ci in range(2):
                            nc.tensor.matmul(
                                po[:], wp_sb[:, ci, co, :],
                                yt_sb[:, ci, j * 512:(j + 1) * 512],
                                start=(ci == 0), stop=False)
                        nc.tensor.matmul(po[:], lbp_sb[:, co, :],
                                         up_sb[:, j * 512:(j + 1) * 512],
                                         start=False, stop=True)
                        oq = wk.tile([P, 512], F32, tag="oq",
                                     name=f"oq{j}_{co}")
                        nc.vector.tensor_scalar_add(oq[:], po[:], 0.0)
                        nc.sync.dma_start(
                            out=out_d[co * P:(co + 1) * P,
                                      j * 512:(j + 1) * 512],
                            in_=oq[:])
                    proj_queue.append(p6)

            for j in range(T5):
                for h in range(HPC):
                    fq = h // 2
                    fk = 2 + h // 2
                    pqb = (h % 2) * HD
                    ni = 4 * (j + 1)
                    npair = ni // 2
                    pav = psav.tile([HD + 1, 512], F32, tag="psav",
                                    name=f"pav{h}_{j}")
                    at_tiles = {}
                    for p in range(npair + 1):
                        if p < npair:
                            pst = pss.tile([P, 2, 512], F32, tag="pss",
                                           name=f"pst{h}_{j}_{p}")
                            qt = qk_sb[pqb:pqb + HD, fq,
                                       j * 512:(j + 1) * 512]
                            for d in (0, 1):
                                i = 2 * p + d
                                kt = qk_sb[pqb:pqb + HD, fk,
                                           i * P:(i + 1) * P]
                                nc.tensor.matmul(pst[:, d, :], kt, qt,
                                                 start=True, stop=True)
                            at = ap_.tile([P, 2, 512], BF16, tag="att",
                                          name=f"at{h}_{j}_{p}")
                            nc.scalar.activation(
                                at[:], pst[:],
                                mybir.ActivationFunctionType.Exp,
                                scale=0.125)
                            for d in (0, 1):
                                a = 2 * p + d - 4 * j
                                if a >= 0:
                                    nc.vector.tensor_tensor(
                                        at[:, d, :], at[:, d, :],
                                        masks[:, 384 - 128 * a:896 - 128 * a],
                                        mybir.AluOpType.mult)
                            at_tiles[p] = at
                        if p == 1 and flush_queue:
                            emit_flush()
                        if p >= 1:
                            q = p - 1
                            at = at_tiles.pop(q)
                            for d in (0, 1):
                                i = 2 * q + d
                                nc.tensor.matmul(pav[:], v_sb[:, i, h, :],
                                                 at[:, d, :],
                                                 start=(i == 0),
                                                 stop=(i == ni - 1))
                        if p >= 1 and proj_queue and (h >= 1 or j >= 2):
                            proj_queue.pop(0)()
                    flush_queue.append((pav, h, j))
                make_proj(j)

            while flush_queue:
                emit_flush()
            while proj_queue:
                proj_queue.pop(0)()

    nc.compile()
    return nc


def _shard_inputs(x, w_attn, b_attn, lora_a_attn, lora_b_attn, w_proj, b_proj,
                  lora_a_proj, lora_b_proj):
    import ml_dtypes
    bf16 = ml_dtypes.bfloat16
    f32 = np.float32
    x = np.asarray(x, f32)
    w_attn = np.asarray(w_attn, f32)
    b_attn = np.asarray(b_attn, f32)
    lora_a_attn = np.asarray(lora_a_attn, f32)
    lora_b_attn = np.asarray(lora_b_attn, f32)
    w_proj = np.asarray(w_proj, f32)
    b_proj = np.asarray(b_proj, f32)
    lora_a_proj = np.asarray(lora_a_proj, f32)
    lora_b_proj = np.asarray(lora_b_proj, f32)

    laa_t = np.ascontiguousarray(lora_a_attn.T).astype(bf16)      # (C, R)
    # masks[p, z] = 1.0 if z >= p + 384 else 0.0
    pp, zz = np.meshgrid(np.arange(P), np.arange(896), indexing="ij")
    masks = (zz >= pp + 384).astype(bf16)
    onesr = np.ones((1, T), bf16)
    vones = np.ones((P, T1 * HPC), bf16)
    in_maps = []
    for core in range(N_CORES):
        b = core // GPB
        heads = [(core % GPB) * HPC + k for k in range(HPC)]
        q_idx = np.concatenate([np.arange(h * HD, (h + 1) * HD) for h in heads])
        k_idx = q_idx + C
        v_idx = q_idx + 2 * C
        qk_idx = np.concatenate([q_idx, k_idx])
        wqk_t = np.ascontiguousarray(w_attn[qk_idx].T).astype(bf16)  # (C, 512)
        wv_t = np.ascontiguousarray(w_attn[v_idx].T).astype(bf16)    # (C, 256)
        lbaqk = np.concatenate(
            [(lora_b_attn[qk_idx] * LORA_SCALE).T, b_attn[qk_idx][None, :]],
            0).astype(bf16)                                          # (R+1, 512)
        lbav = np.concatenate(
            [(lora_b_attn[v_idx] * LORA_SCALE).T, b_attn[v_idx][None, :]],
            0).astype(bf16)                                          # (R+1, 256)
        wp_t = np.ascontiguousarray(w_proj[:, q_idx].T).astype(bf16)  # (256, C)
        lap_t = np.ascontiguousarray(lora_a_proj[:, q_idx].T).astype(bf16)
        bp = b_proj if core % GPB == 0 else np.zeros((C,), f32)
        lbp = np.concatenate(
            [(lora_b_proj * LORA_SCALE).T, bp[None, :]], 0).astype(bf16)
        in_maps.append({
            "xt": np.ascontiguousarray(x[b].T).astype(bf16),
            "wqk": wqk_t, "wv": wv_t,
            "laa": laa_t, "lbaqk": np.ascontiguousarray(lbaqk),
            "lbav": np.ascontiguousarray(lbav),
            "wp": wp_t, "lap": lap_t, "lbp": np.ascontiguousarray(lbp),
            "masks": masks, "onesr": onesr, "vones": vones,
        })
    return in_maps


def kernel(x, w_attn, b_attn, lora_a_attn, lora_b_attn, w_proj, b_proj,
           lora_a_proj, lora_b_proj, n_head):
    global LAST_RESULTS
    assert int(n_head) == H
    if "nc" not in _CACHE:
        _CACHE["nc"] = build()
    nc = _CACHE["nc"]
    in_maps = _shard_inputs(x, w_attn, b_attn, lora_a_attn, lora_b_attn,
                            w_proj, b_proj, lora_a_proj, lora_b_proj)
    res = run_bass_kernel_spmd(
        nc, in_maps, core_ids=list(range(N_CORES)),
        trace=bool(os.environ.get("BASS_KERNEL_TRACE")))
    LAST_RESULTS = res
    out = np.zeros((B, C, T), np.float32)
    for core in range(N_CORES):
        out[core // GPB] += res.results[core]["out"]
    return np.ascontiguousarray(out.transpose(0, 2, 1))
